# revision 33
# baseline (speedup 1.0000x reference)
"""Trainium2 Bass kernel for nn_MinervaEnhancedLossV3 (c-pent layout).

Contract: kernel(**inputs) takes FULL unsharded inputs (B=2048), shards
batch-wise across 8 NeuronCores (256 batches each), runs one SPMD Bass
program, and combines per-batch partial statistics on the host.

Device layout (per core): 4 groups of 64 batches; tile [128, (c5, s)]
with partition p = ch*64 + b (ch = channel half), free = c5 in 0..5
(channel c = 5*ch + c5) x s in HW. Every pred DMA row is contiguous
DRAM; ch=0 rows issue via HWDGE (nc.sync) and ch=1 via SWDGE
(nc.gpsimd) -- the halves hit disjoint SDMA engine sets, which lifts
the effective HBM rate ~25%.

Processing is pipelined in 4 rounds (2 supergroups x 2 s-phases of
1152), each round:
  E    = f16(exp(x))                            ACT
  cmp  = [trep == iotaF] (bcast-AP TT), me = cmp*E      DVE 2x
  mh   = max over c5 (free-dim folds); maxg = cross-half max via
         partition-shift SBUF->SBUF DMA + TT           DVE + SWDGE
  sumexp, epv = one-hot matmuls over (ch, c5), 2 groups packed to
         128 PSUM rows                                  PE
  chain: ln_s, ln_pv (ACT); ce = sub (GPSIMD); pt = exp(-ce);
         u = relu(1-pt); lu = ln(u); p25 = exp(2.5*lu)  ACT
  accums: fs += p25*ce; eq = [epv >= maxg]; eqc += eq; iou += eq*sw
Host: unique colors / transitions / [t==inputs] counts / copy-penalty
  resolve / bonuses / nan guard -- all numpy on the raw inputs.
  targets are pre-cast to f16 host-side (exact for 0..9) so the device
  loads them directly.
"""

import os
from contextlib import ExitStack

import numpy as np

import concourse.bass as bass
import concourse.bacc as bacc
import concourse.tile as tile
import concourse.mybir as mybir
from concourse.bass_utils import run_bass_kernel_spmd

F16 = mybir.dt.float16
F32 = mybir.dt.float32
I32 = mybir.dt.int32
AF = mybir.ActivationFunctionType
OP = mybir.AluOpType

N_CORES = 8
B_FULL = 2048
C = 10
H = W = 48
HW = H * W                       # 2304
NB = 64                          # batches per group
CH = 2                           # channel halves per partition dim
C5 = 5                           # channels per half (free dim)
FREE = C5 * HW                   # 11520
NG = 4                           # groups per core (256 batches)
NSG = 2                          # supergroups (2 groups each)
B_PC = NG * NB                   # 256
PHASES = ((0, 1152), (1152, 1152))
NQ = 3                           # fs, iou, eqc
NCOLS = NSG * len(PHASES) * NQ   # 12

LAST_EXEC_NS = None


def _spatial_weights():
    cy, cx = H // 2, W // 2
    yy = np.arange(H, dtype=np.float64)[:, None]
    xx = np.arange(W, dtype=np.float64)[None, :]
    dist = np.sqrt((yy - cy) ** 2 + (xx - cx) ** 2)
    md = np.sqrt((H // 2) ** 2 + (W // 2) ** 2)
    return (1.0 + 0.3 * (1.0 - dist / md))               # [H, W] f64


def _col(sg, ph, q):
    return sg * (len(PHASES) * NQ) + ph * NQ + q


def build_nc(finalize=True):
    nc = bacc.Bacc(trn_type="TRN2") if finalize else bass.Bass(trn_type="TRN2")

    pred_in = nc.dram_tensor("pred_in", [B_PC, C, HW], F32, kind="ExternalInput")
    t_in = nc.dram_tensor("t_in", [B_PC, HW], F16, kind="ExternalInput")
    out_cols = nc.dram_tensor("out_cols", [128, NCOLS], F32, kind="ExternalOutput")

    # constants
    sw = np.repeat(_spatial_weights().reshape(1, HW), 128, axis=0).astype(np.float16)
    sw_const = nc.inline_tensor(sw, name="sw_const")
    DPc = 1152
    iotaF_np = np.zeros((128, C5 * DPc), dtype=np.float16)
    for ch in range(CH):
        for c5 in range(C5):
            iotaF_np[ch * NB:(ch + 1) * NB, c5 * DPc:(c5 + 1) * DPc] = 5 * ch + c5
    iota_const = nc.inline_tensor(iotaF_np, name="iota_const")
    lhs_np = np.zeros((128, 2 * 128), dtype=np.float16)
    for gl in range(2):
        for ch in range(CH):
            for b in range(NB):
                lhs_np[ch * NB + b, gl * 128 + gl * NB + b] = 1.0
    lhs_const = nc.inline_tensor(lhs_np, name="lhs_const")

    with tile.TileContext(nc) as tc, ExitStack() as es:
        _emit(es, tc, nc, pred_in, t_in, out_cols, sw_const, iota_const, lhs_const)
    if finalize:
        nc.finalize()
    return nc


def _emit(es, tc, nc, pred_in, t_in, out_cols, sw_const, iota_const, lhs_const):
    dma = nc.sync.dma_start
    tt = nc.vector.tensor_tensor
    ts = nc.vector.tensor_scalar
    act = nc.scalar.activation
    gdma = nc.gpsimd.dma_start

    singles = es.enter_context(tc.tile_pool(name="singles", bufs=1))
    xpool = es.enter_context(tc.tile_pool(name="xpool", bufs=2))
    epool = es.enter_context(tc.tile_pool(name="epool", bufs=2))
    mepool = es.enter_context(tc.tile_pool(name="mepool", bufs=2))
    fpool = es.enter_context(tc.tile_pool(name="fpool", bufs=1))
    tpool = es.enter_context(tc.tile_pool(name="tpool", bufs=2))
    mxpool = es.enter_context(tc.tile_pool(name="mxpool", bufs=2))
    mgpool = es.enter_context(tc.tile_pool(name="mgpool", bufs=2))
    chain = es.enter_context(tc.tile_pool(name="chain", bufs=1))
    psum = es.enter_context(tc.tile_pool(name="psum", bufs=1, space="PSUM"))

    DP = 1152                     # phase width in s
    HDP = DP // 2
    pbase = pred_in[:, :, :]

    sw_t = singles.tile([128, HW], F16)
    iotaF = singles.tile([128, C5 * DP], F16)
    lhs_t = singles.tile([128, 2 * 128], F16)
    colstage = singles.tile([128, NCOLS], F32)
    nc.vector.memset(colstage[:], 0.0)

    first_const = [True]

    def load_consts():
        dma(out=sw_t[:], in_=sw_const[:, :])
        dma(out=iotaF[:], in_=iota_const[:, :])
        dma(out=lhs_t[:], in_=lhs_const[:, :])
        first_const[0] = False

    treps = {}

    for sg in range(NSG):
        for ph, (d0, D) in enumerate(PHASES):
            # ---- per-group: load x (2 half-chunks), exp, folds, maxg ----
            e_ts = []
            me_ts = []
            maxg_sg = mgpool.tile([128, DP], F16, tag="maxg_sg")
            for gl in range(2):
                g = sg * 2 + gl
                e_t = epool.tile([128, C5 * DP], F16, tag=f"e{gl}")
                goff = g * NB * (C * HW)
                for hk in range(2):
                    x_t = xpool.tile([128, C5 * HDP], F32, tag="x")
                    for ch in range(CH):
                        src = bass.AP(tensor=pbase.tensor,
                                      offset=goff + ch * FREE + d0 + hk * HDP,
                                      ap=[[C * HW, NB], [HW, C5], [1, HDP]])
                        eng = gdma if ch == 1 else dma
                        eng(out=x_t[ch * NB:(ch + 1) * NB, :], in_=src)
                    if first_const[0]:
                        load_consts()
                    if ph == 0 and gl == 0 and hk == 0:
                        t16 = tpool.tile([128, HW], F16, tag="t16")
                        dma(out=t16[:], in_=t_in[sg * 128:(sg + 1) * 128, :])
                        for gg in range(2):
                            trep = tpool.tile([128, HW], F16, tag=f"trep{gg}")
                            gdma(out=trep[0:NB, :],
                                 in_=t16[gg * NB:(gg + 1) * NB, :])
                            gdma(out=trep[NB:128, :],
                                 in_=t16[gg * NB:(gg + 1) * NB, :])
                            treps[(sg, gg)] = trep
                    edst = bass.AP(tensor=e_t.tensor,
                                   offset=e_t.offset + hk * HDP,
                                   ap=[e_t.ap[0], [DP, C5], [1, HDP]])
                    act(edst, x_t[:], AF.Exp)

                # masks into me (cmp via broadcast-AP + iotaF, then mult)
                me_t = mepool.tile([128, C5 * DP], F16, tag=f"me{gl}")
                trsl = treps[(sg, gl)][:, d0:d0 + D]
                trb = bass.AP(tensor=trsl.tensor, offset=trsl.offset,
                              ap=[trsl.ap[0], [0, C5], [1, D]])
                tt(out=me_t[:], in0=trb, in1=iotaF[:], op=OP.is_equal)
                tt(out=me_t[:], in0=me_t[:], in1=e_t[:], op=OP.mult)

                # folds: per-half max over c5
                fsc = fpool.tile([128, 2 * DP], F16, tag="fsc")
                tt(out=fsc[:], in0=e_t[:, 0:2 * DP],
                   in1=e_t[:, 2 * DP:4 * DP], op=OP.max)
                mh = mxpool.tile([128, DP], F16, tag="mh")
                tt(out=mh[:], in0=fsc[:, 0:DP], in1=fsc[:, DP:2 * DP],
                   op=OP.max)
                tt(out=mh[:], in0=mh[:], in1=e_t[:, 4 * DP:5 * DP], op=OP.max)
                mhs = mxpool.tile([NB, DP], F16, tag="mhs")
                gdma(out=mhs[:], in_=mh[NB:128, :])
                maxg = mxpool.tile([NB, DP], F16, tag="maxg")
                tt(out=maxg[:], in0=mh[0:NB, :], in1=mhs[:], op=OP.max)
                gdma(out=maxg_sg[gl * NB:(gl + 1) * NB, :], in_=maxg[:])
                e_ts.append(e_t)
                me_ts.append(me_t)

            # ---- PE passes ----
            sum_ps = psum.tile([128, DP], F32, tag="sum")
            epv_ps = psum.tile([128, DP], F32, tag="epv")
            for ps, tiles in ((sum_ps, e_ts), (epv_ps, me_ts)):
                for gl in range(2):
                    lw = lhs_t[:, gl * 128:(gl + 1) * 128]
                    for c5 in range(C5):
                        first = (gl == 0 and c5 == 0)
                        last = (gl == 1 and c5 == C5 - 1)
                        for k0 in range(0, D, 512):
                            kn = min(512, D - k0)
                            nc.tensor.matmul(
                                ps[:, k0:k0 + kn], lw,
                                tiles[gl][:, c5 * DP + k0:c5 * DP + k0 + kn],
                                start=first, stop=last)

            # ---- chain ----
            ln_s = chain.tile([128, DP], F32, tag="ln_s")
            act(ln_s[:], sum_ps[:], AF.Ln)
            ln_pv = chain.tile([128, DP], F32, tag="ln_pv")
            act(ln_pv[:], epv_ps[:], AF.Ln)
            ce = chain.tile([128, DP], F16, tag="ce")
            nc.gpsimd.tensor_tensor(out=ce[:], in0=ln_s[:], in1=ln_pv[:],
                                    op=OP.subtract)
            # compare in ln-space: frees the epv PSUM at ln_pv (ACT, early)
            # instead of at the DVE compare (late); ln is monotone and maps
            # equal f16 inputs identically, so the eq bit is preserved
            lmax = chain.tile([128, DP], F32, tag="lmax")
            act(lmax[:], maxg_sg[:], AF.Ln)
            eq = chain.tile([128, DP], F16, tag="eq")
            tt(out=eq[:], in0=ln_pv[:], in1=lmax[:], op=OP.is_ge)
            iscr = chain.tile([128, DP], F16, tag="scr")
            nc.vector.scalar_tensor_tensor(
                out=iscr[:], in0=eq[:], scalar=0.0, in1=sw_t[:, d0:d0 + D],
                op0=OP.bypass, op1=OP.mult,
                accum_out=colstage[:, _col(sg, ph, 1):_col(sg, ph, 1) + 1])
            ts(out=eq[:], in0=eq[:], scalar1=1.0, scalar2=None,
               op0=OP.mult, op1=OP.add,
               accum_out=colstage[:, _col(sg, ph, 2):_col(sg, ph, 2) + 1])

            pt = chain.tile([128, DP], F16, tag="pt")
            act(pt[:], ce[:], AF.Exp, scale=-1.0)
            u = chain.tile([128, DP], F16, tag="u")
            act(u[:], pt[:], AF.Relu, bias=1.0, scale=-1.0)
            lu = chain.tile([128, DP], F16, tag="lu")
            act(lu[:], u[:], AF.Ln)
            p25 = chain.tile([128, DP], F16, tag="p25")
            act(p25[:], lu[:], AF.Exp, scale=2.5)
            fscr = chain.tile([128, DP], F16, tag="scr")
            nc.vector.scalar_tensor_tensor(
                out=fscr[:], in0=p25[:], scalar=0.0, in1=ce[:],
                op0=OP.bypass, op1=OP.mult,
                accum_out=colstage[:, _col(sg, ph, 0):_col(sg, ph, 0) + 1])

    dma(out=out_cols[:, :], in_=colstage[:])


_NC_CACHE = {}


def _get_nc():
    if "nc" not in _NC_CACHE:
        _NC_CACHE["nc"] = build_nc()
    return _NC_CACHE["nc"]


def _combine(res_list, pred, targets, inputs_arr, sf, ps, rd):
    B = pred.shape[0]
    t2 = targets.reshape(B, HW)

    fs = np.zeros(B, np.float64)
    iou_s = np.zeros(B, np.float64)
    eqc = np.zeros(B, np.float64)
    for core, r in enumerate(res_list):
        cols = r["out_cols"].astype(np.float64)          # [128, NCOLS]
        for sg in range(NSG):
            rows = np.arange(128)
            bidx = core * B_PC + sg * 128 + rows
            f = i = e = 0.0
            f = sum(cols[:, _col(sg, ph, 0)] for ph in range(len(PHASES)))
            i = sum(cols[:, _col(sg, ph, 1)] for ph in range(len(PHASES)))
            e = sum(cols[:, _col(sg, ph, 2)] for ph in range(len(PHASES)))
            fs[bidx] = f
            iou_s[bidx] = i
            eqc[bidx] = np.rint(e)

    # host-side stats from raw int inputs
    t3 = targets.reshape(B, H, W)
    trans = (t3[:, :, 1:] != t3[:, :, :-1]).sum((1, 2)) + \
            (t3[:, 1:, :] != t3[:, :-1, :]).sum((1, 2))
    off = (np.arange(B, dtype=np.int64)[:, None] * C)
    uniq = np.bincount((t2 + off).ravel(), minlength=B * C) \
        .reshape(B, C).astype(bool).sum(1)
    dcnt = (t2 == inputs_arr.reshape(B, HW)).sum(1)

    w = np.where(uniq > 4, 1.3, 1.0) * np.where(trans > W, 1.2, 1.0)
    focal = (fs * w).sum() / (B * HW)

    sw64 = _spatial_weights()
    SW = sw64.sum()
    strict = eqc == HW
    iou = iou_s / SW
    ut = 0.85 * iou + 0.15 * strict
    ut_mean = ut.mean()
    exact_bonus = max(-ut_mean * 5.0, -5.0)

    cand = np.where(eqc == dcnt)[0]
    copy = np.zeros(B, np.float64)
    if cand.size:
        pr = pred.reshape(B, C, HW)
        am = pr[cand].argmax(1)
        copy[cand] = (am == inputs_arr.reshape(B, HW)[cand]).all(1)
    transform_penalty = copy.mean() * 0.5

    sf_mean = sf.astype(np.float64).mean()
    creativity = 1.0 / (1.0 + np.exp(-sf_mean)) * 0.1
    strategic = ps.astype(np.float64).mean() * 0.1
    multi = rd.astype(np.float64).mean() * 0.1
    complexity = ut_mean * (HW / 1225.0) * 0.1

    total = (focal + transform_penalty + exact_bonus
             - creativity - strategic - multi - complexity)
    if np.isnan(total) or np.isinf(total):
        total = min(focal, 10.0)
    return np.float32(total)


def kernel(pred, strategic_features, planning_score, reasoning_depth,
           targets, inputs):
    global LAST_EXEC_NS
    pred = np.ascontiguousarray(np.asarray(pred, dtype=np.float32))
    targets = np.ascontiguousarray(np.asarray(targets, dtype=np.int32))
    inputs_arr = np.ascontiguousarray(np.asarray(inputs, dtype=np.int32))
    sf = np.asarray(strategic_features, dtype=np.float32)
    ps = np.asarray(planning_score, dtype=np.float32)
    rd = np.asarray(reasoning_depth, dtype=np.float32)

    nc = _get_nc()
    t16_full = targets.astype(np.float16)
    in_maps = []
    for core in range(N_CORES):
        sl = slice(core * B_PC, (core + 1) * B_PC)
        in_maps.append({
            "pred_in": pred[sl].reshape(B_PC, C, HW),
            "t_in": t16_full[sl].reshape(B_PC, HW),
        })

    trace = os.environ.get("BASSLOSS_TRACE", "0") == "1"
    res = run_bass_kernel_spmd(nc, in_maps, list(range(N_CORES)), trace=trace)
    LAST_EXEC_NS = res.exec_time_ns

    return _combine(res.results, pred, targets, inputs_arr, sf, ps, rd)


if __name__ == "__main__":
    d = np.load("/root/problem/inputs_cache.npz")
    out = kernel(**{k: d[k] for k in d.files})
    print("kernel out:", out, " exec_ns:", LAST_EXEC_NS)


# revision 34
# speedup vs baseline: 1.0814x; 1.0814x over previous
"""Trainium2 Bass kernel for nn_MinervaEnhancedLossV3 (c-pent layout).

Contract: kernel(**inputs) takes FULL unsharded inputs (B=2048), shards
batch-wise across 8 NeuronCores (256 batches each), runs one SPMD Bass
program, and combines per-batch partial statistics on the host.

Device layout (per core): 4 groups of 64 batches; tile [128, (c5, s)]
with partition p = ch*64 + b (ch = channel half), free = c5 in 0..5
(channel c = 5*ch + c5) x s in HW. Every pred DMA row is contiguous
DRAM; ch=0 rows issue via HWDGE (nc.sync) and ch=1 via SWDGE
(nc.gpsimd) -- the halves hit disjoint SDMA engine sets, which lifts
the effective HBM rate ~25%.

Processing is pipelined in 4 rounds (2 supergroups x 2 s-phases of
1152), each round:
  E    = f16(exp(x))                            ACT
  cmp  = [trep == iotaF] (bcast-AP TT), me = cmp*E      DVE 2x
  mh   = max over c5 (free-dim folds); maxg = cross-half max via
         partition-shift SBUF->SBUF DMA + TT           DVE + SWDGE
  sumexp, epv = one-hot matmuls over (ch, c5), 2 groups packed to
         128 PSUM rows                                  PE
  chain: ln_s, ln_pv (ACT); ce = sub (GPSIMD); pt = exp(-ce);
         u = relu(1-pt); lu = ln(u); p25 = exp(2.5*lu)  ACT
  accums: fs += p25*ce; eq = [epv >= maxg]; eqc += eq; iou += eq*sw
Host: unique colors / transitions / [t==inputs] counts / copy-penalty
  resolve / bonuses / nan guard -- all numpy on the raw inputs.
  targets are pre-cast to f16 host-side (exact for 0..9) so the device
  loads them directly.
"""

import os
from contextlib import ExitStack

import numpy as np

import concourse.bass as bass
import concourse.bacc as bacc
import concourse.tile as tile
import concourse.mybir as mybir
from concourse.bass_utils import run_bass_kernel_spmd

F16 = mybir.dt.float16
F32 = mybir.dt.float32
I32 = mybir.dt.int32
AF = mybir.ActivationFunctionType
OP = mybir.AluOpType

N_CORES = 8
B_FULL = 2048
C = 10
H = W = 48
HW = H * W                       # 2304
NB = 64                          # batches per group
CH = 2                           # channel halves per partition dim
C5 = 5                           # channels per half (free dim)
FREE = C5 * HW                   # 11520
NG = 4                           # groups per core (256 batches)
NSG = 2                          # supergroups (2 groups each)
B_PC = NG * NB                   # 256
PHASES = ((0, 1152), (1152, 1152))
NQ = 3                           # fs, iou, eqc
NCOLS = NSG * len(PHASES) * NQ   # 12

LAST_EXEC_NS = None


def _spatial_weights():
    cy, cx = H // 2, W // 2
    yy = np.arange(H, dtype=np.float64)[:, None]
    xx = np.arange(W, dtype=np.float64)[None, :]
    dist = np.sqrt((yy - cy) ** 2 + (xx - cx) ** 2)
    md = np.sqrt((H // 2) ** 2 + (W // 2) ** 2)
    return (1.0 + 0.3 * (1.0 - dist / md))               # [H, W] f64


def _col(sg, ph, q):
    return sg * (len(PHASES) * NQ) + ph * NQ + q


def build_nc(finalize=True):
    nc = bacc.Bacc(trn_type="TRN2") if finalize else bass.Bass(trn_type="TRN2")

    pred_in = nc.dram_tensor("pred_in", [B_PC, C, HW], F32, kind="ExternalInput")
    t_in = nc.dram_tensor("t_in", [B_PC, HW], F16, kind="ExternalInput")
    out_cols = nc.dram_tensor("out_cols", [128, NCOLS], F32, kind="ExternalOutput")

    # constants
    sw = np.repeat(_spatial_weights().reshape(1, HW), 128, axis=0).astype(np.float16)
    sw_const = nc.inline_tensor(sw, name="sw_const")
    DPc = 1152
    iotaF_np = np.zeros((128, C5 * DPc), dtype=np.float16)
    for ch in range(CH):
        for c5 in range(C5):
            iotaF_np[ch * NB:(ch + 1) * NB, c5 * DPc:(c5 + 1) * DPc] = 5 * ch + c5
    iota_const = nc.inline_tensor(iotaF_np, name="iota_const")
    lhs_np = np.zeros((128, 2 * 128), dtype=np.float16)
    for gl in range(2):
        for ch in range(CH):
            for b in range(NB):
                lhs_np[ch * NB + b, gl * 128 + gl * NB + b] = 1.0
    lhs_const = nc.inline_tensor(lhs_np, name="lhs_const")

    with tile.TileContext(nc) as tc, ExitStack() as es:
        _emit(es, tc, nc, pred_in, t_in, out_cols, sw_const, iota_const, lhs_const)
    if finalize:
        nc.finalize()
    return nc


def _emit(es, tc, nc, pred_in, t_in, out_cols, sw_const, iota_const, lhs_const):
    dma = nc.sync.dma_start
    tt = nc.vector.tensor_tensor
    ts = nc.vector.tensor_scalar
    act = nc.scalar.activation
    gdma = nc.gpsimd.dma_start

    singles = es.enter_context(tc.tile_pool(name="singles", bufs=1))
    xpool = es.enter_context(tc.tile_pool(name="xpool", bufs=2))
    epool = es.enter_context(tc.tile_pool(name="epool", bufs=2))
    mepool = es.enter_context(tc.tile_pool(name="mepool", bufs=2))
    fpool = es.enter_context(tc.tile_pool(name="fpool", bufs=2))
    tpool = es.enter_context(tc.tile_pool(name="tpool", bufs=2))
    mxpool = es.enter_context(tc.tile_pool(name="mxpool", bufs=2))
    mgpool = es.enter_context(tc.tile_pool(name="mgpool", bufs=2))
    chain = es.enter_context(tc.tile_pool(name="chain", bufs=1))
    psum = es.enter_context(tc.tile_pool(name="psum", bufs=1, space="PSUM"))

    DP = 1152                     # phase width in s
    HDP = DP // 2
    pbase = pred_in[:, :, :]

    sw_t = singles.tile([128, HW], F16)
    iotaF = singles.tile([128, C5 * DP], F16)
    lhs_t = singles.tile([128, 2 * 128], F16)
    colstage = singles.tile([128, NCOLS], F32)
    nc.vector.memset(colstage[:], 0.0)

    first_const = [True]

    def load_consts():
        dma(out=sw_t[:], in_=sw_const[:, :])
        dma(out=iotaF[:], in_=iota_const[:, :])
        dma(out=lhs_t[:], in_=lhs_const[:, :])
        first_const[0] = False

    treps = {}

    for sg in range(NSG):
        for ph, (d0, D) in enumerate(PHASES):
            # ---- per-group: load x (2 half-chunks), exp, folds, maxg ----
            e_ts = []
            me_ts = []
            maxg_sg = mgpool.tile([128, DP], F16, tag="maxg_sg")
            for gl in range(2):
                g = sg * 2 + gl
                e_t = epool.tile([128, C5 * DP], F16, tag=f"e{gl}")
                goff = g * NB * (C * HW)
                for hk in range(2):
                    x_t = xpool.tile([128, C5 * HDP], F32, tag="x")
                    for ch in range(CH):
                        src = bass.AP(tensor=pbase.tensor,
                                      offset=goff + ch * FREE + d0 + hk * HDP,
                                      ap=[[C * HW, NB], [HW, C5], [1, HDP]])
                        eng = gdma if ch == 1 else dma
                        eng(out=x_t[ch * NB:(ch + 1) * NB, :], in_=src)
                    if first_const[0]:
                        load_consts()
                    if ph == 0 and gl == 0 and hk == 0:
                        t16 = tpool.tile([128, HW], F16, tag="t16")
                        dma(out=t16[:], in_=t_in[sg * 128:(sg + 1) * 128, :])
                        for gg in range(2):
                            trep = tpool.tile([128, HW], F16, tag=f"trep{gg}")
                            gdma(out=trep[0:NB, :],
                                 in_=t16[gg * NB:(gg + 1) * NB, :])
                            gdma(out=trep[NB:128, :],
                                 in_=t16[gg * NB:(gg + 1) * NB, :])
                            treps[(sg, gg)] = trep
                    edst = bass.AP(tensor=e_t.tensor,
                                   offset=e_t.offset + hk * HDP,
                                   ap=[e_t.ap[0], [DP, C5], [1, HDP]])
                    act(edst, x_t[:], AF.Exp)

                # masks into me (cmp via broadcast-AP + iotaF, then mult)
                me_t = mepool.tile([128, C5 * DP], F16, tag=f"me{gl}")
                trsl = treps[(sg, gl)][:, d0:d0 + D]
                trb = bass.AP(tensor=trsl.tensor, offset=trsl.offset,
                              ap=[trsl.ap[0], [0, C5], [1, D]])
                tt(out=me_t[:], in0=trb, in1=iotaF[:], op=OP.is_equal)
                tt(out=me_t[:], in0=me_t[:], in1=e_t[:], op=OP.mult)

                # folds: per-half max over c5
                fsc = fpool.tile([128, 2 * DP], F16, tag="fsc")
                tt(out=fsc[:], in0=e_t[:, 0:2 * DP],
                   in1=e_t[:, 2 * DP:4 * DP], op=OP.max)
                mh = mxpool.tile([128, DP], F16, tag="mh")
                tt(out=mh[:], in0=fsc[:, 0:DP], in1=fsc[:, DP:2 * DP],
                   op=OP.max)
                tt(out=mh[:], in0=mh[:], in1=e_t[:, 4 * DP:5 * DP], op=OP.max)
                mhs = mxpool.tile([NB, DP], F16, tag="mhs")
                gdma(out=mhs[:], in_=mh[NB:128, :])
                maxg = mxpool.tile([NB, DP], F16, tag="maxg")
                tt(out=maxg[:], in0=mh[0:NB, :], in1=mhs[:], op=OP.max)
                gdma(out=maxg_sg[gl * NB:(gl + 1) * NB, :], in_=maxg[:])
                e_ts.append(e_t)
                me_ts.append(me_t)

            # ---- PE passes ----
            sum_ps = psum.tile([128, DP], F32, tag="sum")
            epv_ps = psum.tile([128, DP], F32, tag="epv")
            for ps, tiles in ((sum_ps, e_ts), (epv_ps, me_ts)):
                for gl in range(2):
                    lw = lhs_t[:, gl * 128:(gl + 1) * 128]
                    for c5 in range(C5):
                        first = (gl == 0 and c5 == 0)
                        last = (gl == 1 and c5 == C5 - 1)
                        for k0 in range(0, D, 512):
                            kn = min(512, D - k0)
                            nc.tensor.matmul(
                                ps[:, k0:k0 + kn], lw,
                                tiles[gl][:, c5 * DP + k0:c5 * DP + k0 + kn],
                                start=first, stop=last)

            # ---- chain ----
            ln_s = chain.tile([128, DP], F32, tag="ln_s")
            act(ln_s[:], sum_ps[:], AF.Ln)
            ln_pv = chain.tile([128, DP], F32, tag="ln_pv")
            act(ln_pv[:], epv_ps[:], AF.Ln)
            ce = chain.tile([128, DP], F16, tag="ce")
            nc.gpsimd.tensor_tensor(out=ce[:], in0=ln_s[:], in1=ln_pv[:],
                                    op=OP.subtract)
            eq = chain.tile([128, DP], F16, tag="eq")
            tt(out=eq[:], in0=epv_ps[:], in1=maxg_sg[:], op=OP.is_ge)
            iscr = chain.tile([128, DP], F16, tag="scr")
            nc.vector.scalar_tensor_tensor(
                out=iscr[:], in0=eq[:], scalar=0.0, in1=sw_t[:, d0:d0 + D],
                op0=OP.bypass, op1=OP.mult,
                accum_out=colstage[:, _col(sg, ph, 1):_col(sg, ph, 1) + 1])
            ts(out=eq[:], in0=eq[:], scalar1=1.0, scalar2=None,
               op0=OP.mult, op1=OP.add,
               accum_out=colstage[:, _col(sg, ph, 2):_col(sg, ph, 2) + 1])

            pt = chain.tile([128, DP], F16, tag="pt")
            act(pt[:], ce[:], AF.Exp, scale=-1.0)
            u = chain.tile([128, DP], F16, tag="u")
            act(u[:], pt[:], AF.Relu, bias=1.0, scale=-1.0)
            lu = chain.tile([128, DP], F16, tag="lu")
            act(lu[:], u[:], AF.Ln)
            p25 = chain.tile([128, DP], F16, tag="p25")
            act(p25[:], lu[:], AF.Exp, scale=2.5)
            fscr = chain.tile([128, DP], F16, tag="scr")
            nc.vector.scalar_tensor_tensor(
                out=fscr[:], in0=p25[:], scalar=0.0, in1=ce[:],
                op0=OP.bypass, op1=OP.mult,
                accum_out=colstage[:, _col(sg, ph, 0):_col(sg, ph, 0) + 1])

    dma(out=out_cols[:, :], in_=colstage[:])


_NC_CACHE = {}


def _get_nc():
    if "nc" not in _NC_CACHE:
        _NC_CACHE["nc"] = build_nc()
    return _NC_CACHE["nc"]


def _combine(res_list, pred, targets, inputs_arr, sf, ps, rd):
    B = pred.shape[0]
    t2 = targets.reshape(B, HW)

    fs = np.zeros(B, np.float64)
    iou_s = np.zeros(B, np.float64)
    eqc = np.zeros(B, np.float64)
    for core, r in enumerate(res_list):
        cols = r["out_cols"].astype(np.float64)          # [128, NCOLS]
        for sg in range(NSG):
            rows = np.arange(128)
            bidx = core * B_PC + sg * 128 + rows
            f = i = e = 0.0
            f = sum(cols[:, _col(sg, ph, 0)] for ph in range(len(PHASES)))
            i = sum(cols[:, _col(sg, ph, 1)] for ph in range(len(PHASES)))
            e = sum(cols[:, _col(sg, ph, 2)] for ph in range(len(PHASES)))
            fs[bidx] = f
            iou_s[bidx] = i
            eqc[bidx] = np.rint(e)

    # host-side stats from raw int inputs
    t3 = targets.reshape(B, H, W)
    trans = (t3[:, :, 1:] != t3[:, :, :-1]).sum((1, 2)) + \
            (t3[:, 1:, :] != t3[:, :-1, :]).sum((1, 2))
    off = (np.arange(B, dtype=np.int64)[:, None] * C)
    uniq = np.bincount((t2 + off).ravel(), minlength=B * C) \
        .reshape(B, C).astype(bool).sum(1)
    dcnt = (t2 == inputs_arr.reshape(B, HW)).sum(1)

    w = np.where(uniq > 4, 1.3, 1.0) * np.where(trans > W, 1.2, 1.0)
    focal = (fs * w).sum() / (B * HW)

    sw64 = _spatial_weights()
    SW = sw64.sum()
    strict = eqc == HW
    iou = iou_s / SW
    ut = 0.85 * iou + 0.15 * strict
    ut_mean = ut.mean()
    exact_bonus = max(-ut_mean * 5.0, -5.0)

    cand = np.where(eqc == dcnt)[0]
    copy = np.zeros(B, np.float64)
    if cand.size:
        pr = pred.reshape(B, C, HW)
        am = pr[cand].argmax(1)
        copy[cand] = (am == inputs_arr.reshape(B, HW)[cand]).all(1)
    transform_penalty = copy.mean() * 0.5

    sf_mean = sf.astype(np.float64).mean()
    creativity = 1.0 / (1.0 + np.exp(-sf_mean)) * 0.1
    strategic = ps.astype(np.float64).mean() * 0.1
    multi = rd.astype(np.float64).mean() * 0.1
    complexity = ut_mean * (HW / 1225.0) * 0.1

    total = (focal + transform_penalty + exact_bonus
             - creativity - strategic - multi - complexity)
    if np.isnan(total) or np.isinf(total):
        total = min(focal, 10.0)
    return np.float32(total)


def kernel(pred, strategic_features, planning_score, reasoning_depth,
           targets, inputs):
    global LAST_EXEC_NS
    pred = np.ascontiguousarray(np.asarray(pred, dtype=np.float32))
    targets = np.ascontiguousarray(np.asarray(targets, dtype=np.int32))
    inputs_arr = np.ascontiguousarray(np.asarray(inputs, dtype=np.int32))
    sf = np.asarray(strategic_features, dtype=np.float32)
    ps = np.asarray(planning_score, dtype=np.float32)
    rd = np.asarray(reasoning_depth, dtype=np.float32)

    nc = _get_nc()
    t16_full = targets.astype(np.float16)
    in_maps = []
    for core in range(N_CORES):
        sl = slice(core * B_PC, (core + 1) * B_PC)
        in_maps.append({
            "pred_in": pred[sl].reshape(B_PC, C, HW),
            "t_in": t16_full[sl].reshape(B_PC, HW),
        })

    trace = os.environ.get("BASSLOSS_TRACE", "0") == "1"
    res = run_bass_kernel_spmd(nc, in_maps, list(range(N_CORES)), trace=trace)
    LAST_EXEC_NS = res.exec_time_ns

    return _combine(res.results, pred, targets, inputs_arr, sf, ps, rd)


if __name__ == "__main__":
    d = np.load("/root/problem/inputs_cache.npz")
    out = kernel(**{k: d[k] for k in d.files})
    print("kernel out:", out, " exec_ns:", LAST_EXEC_NS)


# revision 35
# speedup vs baseline: 1.0843x; 1.0027x over previous
"""Trainium2 Bass kernel for nn_MinervaEnhancedLossV3 (c-pent layout).

Contract: kernel(**inputs) takes FULL unsharded inputs (B=2048), shards
batch-wise across 8 NeuronCores (256 batches each), runs one SPMD Bass
program, and combines per-batch partial statistics on the host.

Device layout (per core): 4 groups of 64 batches; tile [128, (c5, s)]
with partition p = ch*64 + b (ch = channel half), free = c5 in 0..5
(channel c = 5*ch + c5) x s in HW. Every pred DMA row is contiguous
DRAM; ch=0 rows issue via HWDGE (nc.sync) and ch=1 via SWDGE
(nc.gpsimd) -- the halves hit disjoint SDMA engine sets, which lifts
the effective HBM rate ~25%.

Processing is pipelined in 4 rounds (2 supergroups x 2 s-phases of
1152), each round:
  E    = f16(exp(x))                            ACT
  cmp  = [trep == iotaF] (bcast-AP TT), me = cmp*E      DVE 2x
  mh   = max over c5 (free-dim folds); maxg = cross-half max via
         partition-shift SBUF->SBUF DMA + TT           DVE + SWDGE
  sumexp, epv = one-hot matmuls over (ch, c5), 2 groups packed to
         128 PSUM rows                                  PE
  chain: ln_s, ln_pv (ACT); ce = sub (GPSIMD); pt = exp(-ce);
         u = relu(1-pt); lu = ln(u); p25 = exp(2.5*lu)  ACT
  accums: fs += p25*ce; eq = [epv >= maxg]; eqc += eq; iou += eq*sw
Host: unique colors / transitions / [t==inputs] counts / copy-penalty
  resolve / bonuses / nan guard -- all numpy on the raw inputs.
  targets are pre-cast to f16 host-side (exact for 0..9) so the device
  loads them directly.
"""

import os
from contextlib import ExitStack

import numpy as np

import concourse.bass as bass
import concourse.bacc as bacc
import concourse.tile as tile
import concourse.mybir as mybir
from concourse.bass_utils import run_bass_kernel_spmd

F16 = mybir.dt.float16
F32 = mybir.dt.float32
I32 = mybir.dt.int32
AF = mybir.ActivationFunctionType
OP = mybir.AluOpType

N_CORES = 8
B_FULL = 2048
C = 10
H = W = 48
HW = H * W                       # 2304
NB = 64                          # batches per group
CH = 2                           # channel halves per partition dim
C5 = 5                           # channels per half (free dim)
FREE = C5 * HW                   # 11520
NG = 4                           # groups per core (256 batches)
NSG = 2                          # supergroups (2 groups each)
B_PC = NG * NB                   # 256
PHASES = ((0, 1152), (1152, 1152))
NQ = 3                           # fs, iou, eqc
NCOLS = NSG * len(PHASES) * NQ   # 12

LAST_EXEC_NS = None


def _spatial_weights():
    cy, cx = H // 2, W // 2
    yy = np.arange(H, dtype=np.float64)[:, None]
    xx = np.arange(W, dtype=np.float64)[None, :]
    dist = np.sqrt((yy - cy) ** 2 + (xx - cx) ** 2)
    md = np.sqrt((H // 2) ** 2 + (W // 2) ** 2)
    return (1.0 + 0.3 * (1.0 - dist / md))               # [H, W] f64


def _col(sg, ph, q):
    return sg * (len(PHASES) * NQ) + ph * NQ + q


def build_nc(finalize=True):
    nc = bacc.Bacc(trn_type="TRN2") if finalize else bass.Bass(trn_type="TRN2")

    pred_in = nc.dram_tensor("pred_in", [B_PC, C, HW], F32, kind="ExternalInput")
    t_in = nc.dram_tensor("t_in", [B_PC, HW], F16, kind="ExternalInput")
    out_cols = nc.dram_tensor("out_cols", [128, NCOLS], F32, kind="ExternalOutput")

    # constants
    sw = np.repeat(_spatial_weights().reshape(1, HW), 128, axis=0).astype(np.float16)
    sw_const = nc.inline_tensor(sw, name="sw_const")
    DPc = 1152
    iotaF_np = np.zeros((128, C5 * DPc), dtype=np.float16)
    for ch in range(CH):
        for c5 in range(C5):
            iotaF_np[ch * NB:(ch + 1) * NB, c5 * DPc:(c5 + 1) * DPc] = 5 * ch + c5
    iota_const = nc.inline_tensor(iotaF_np, name="iota_const")
    lhs_np = np.zeros((128, 2 * 128), dtype=np.float16)
    for gl in range(2):
        for ch in range(CH):
            for b in range(NB):
                lhs_np[ch * NB + b, gl * 128 + gl * NB + b] = 1.0
    lhs_const = nc.inline_tensor(lhs_np, name="lhs_const")

    with tile.TileContext(nc) as tc, ExitStack() as es:
        _emit(es, tc, nc, pred_in, t_in, out_cols, sw_const, iota_const, lhs_const)
    if finalize:
        nc.finalize()
    return nc


def _emit(es, tc, nc, pred_in, t_in, out_cols, sw_const, iota_const, lhs_const):
    dma = nc.sync.dma_start
    tt = nc.vector.tensor_tensor
    ts = nc.vector.tensor_scalar
    act = nc.scalar.activation
    gdma = nc.gpsimd.dma_start

    singles = es.enter_context(tc.tile_pool(name="singles", bufs=1))
    xpool = es.enter_context(tc.tile_pool(name="xpool", bufs=2))
    epool = es.enter_context(tc.tile_pool(name="epool", bufs=2))
    mepool = es.enter_context(tc.tile_pool(name="mepool", bufs=2))
    fpool = es.enter_context(tc.tile_pool(name="fpool", bufs=2))
    tpool = es.enter_context(tc.tile_pool(name="tpool", bufs=2))
    mxpool = es.enter_context(tc.tile_pool(name="mxpool", bufs=2))
    mgpool = es.enter_context(tc.tile_pool(name="mgpool", bufs=2))
    chain = es.enter_context(tc.tile_pool(name="chain", bufs=1))
    psum = es.enter_context(tc.tile_pool(name="psum", bufs=1, space="PSUM"))

    DP = 1152                     # phase width in s
    HDP = DP // 2
    pbase = pred_in[:, :, :]

    sw_t = singles.tile([128, HW], F16)
    iotaF = singles.tile([128, C5 * DP], F16)
    lhs_t = singles.tile([128, 2 * 128], F16)
    colstage = singles.tile([128, NCOLS], F32)
    nc.vector.memset(colstage[:], 0.0)

    first_const = [True]

    def load_consts():
        dma(out=sw_t[:], in_=sw_const[:, :])
        dma(out=iotaF[:], in_=iota_const[:, :])
        dma(out=lhs_t[:], in_=lhs_const[:, :])
        first_const[0] = False

    treps = {}

    for sg in range(NSG):
        for ph, (d0, D) in enumerate(PHASES):
            # ---- per-group: load x (2 half-chunks), exp, folds, maxg ----
            e_ts = []
            me_ts = []
            maxg_sg = mgpool.tile([128, DP], F16, tag="maxg_sg")
            for gl in range(2):
                g = sg * 2 + gl
                e_t = epool.tile([128, C5 * DP], F16, tag=f"e{gl}")
                goff = g * NB * (C * HW)
                for hk in range(2):
                    x_t = xpool.tile([128, C5 * HDP], F32, tag="x")
                    for ch in range(CH):
                        src = bass.AP(tensor=pbase.tensor,
                                      offset=goff + ch * FREE + d0 + hk * HDP,
                                      ap=[[C * HW, NB], [HW, C5], [1, HDP]])
                        eng = gdma if ch == 1 else dma
                        eng(out=x_t[ch * NB:(ch + 1) * NB, :], in_=src)
                    if first_const[0]:
                        load_consts()
                    if ph == 0 and gl == 0 and hk == 0:
                        t16 = tpool.tile([128, HW], F16, tag="t16")
                        dma(out=t16[:], in_=t_in[sg * 128:(sg + 1) * 128, :])
                        for gg in range(2):
                            trep = tpool.tile([128, HW], F16, tag=f"trep{gg}")
                            gdma(out=trep[0:NB, :],
                                 in_=t16[gg * NB:(gg + 1) * NB, :])
                            gdma(out=trep[NB:128, :],
                                 in_=t16[gg * NB:(gg + 1) * NB, :])
                            treps[(sg, gg)] = trep
                    edst = bass.AP(tensor=e_t.tensor,
                                   offset=e_t.offset + hk * HDP,
                                   ap=[e_t.ap[0], [DP, C5], [1, HDP]])
                    act(edst, x_t[:], AF.Exp)

                # masks into me (cmp via broadcast-AP + iotaF, then mult)
                me_t = mepool.tile([128, C5 * DP], F16, tag=f"me{gl}")
                trsl = treps[(sg, gl)][:, d0:d0 + D]
                trb = bass.AP(tensor=trsl.tensor, offset=trsl.offset,
                              ap=[trsl.ap[0], [0, C5], [1, D]])
                tt(out=me_t[:], in0=trb, in1=iotaF[:], op=OP.is_equal)
                tt(out=me_t[:], in0=me_t[:], in1=e_t[:], op=OP.mult)

                # folds: per-half max over c5
                fsc = fpool.tile([128, 2 * DP], F16, tag="fsc")
                tt(out=fsc[:], in0=e_t[:, 0:2 * DP],
                   in1=e_t[:, 2 * DP:4 * DP], op=OP.max)
                mh = mxpool.tile([128, DP], F16, tag="mh")
                tt(out=mh[:], in0=fsc[:, 0:DP], in1=fsc[:, DP:2 * DP],
                   op=OP.max)
                tt(out=mh[:], in0=mh[:], in1=e_t[:, 4 * DP:5 * DP], op=OP.max)
                mhs = mxpool.tile([NB, DP], F16, tag="mhs")
                gdma(out=mhs[:], in_=mh[NB:128, :])
                maxg = mxpool.tile([NB, DP], F16, tag="maxg")
                tt(out=maxg[:], in0=mh[0:NB, :], in1=mhs[:], op=OP.max)
                gdma(out=maxg_sg[gl * NB:(gl + 1) * NB, :], in_=maxg[:])
                e_ts.append(e_t)
                me_ts.append(me_t)

            # ---- PE passes ----
            sum_ps = psum.tile([128, DP], F32, tag="sum")
            epv_ps = psum.tile([128, DP], F32, tag="epv")
            for ps, tiles in ((sum_ps, e_ts), (epv_ps, me_ts)):
                for gl in range(2):
                    lw = lhs_t[:, gl * 128:(gl + 1) * 128]
                    for c5 in range(C5):
                        first = (gl == 0 and c5 == 0)
                        last = (gl == 1 and c5 == C5 - 1)
                        for k0 in range(0, D, 512):
                            kn = min(512, D - k0)
                            nc.tensor.matmul(
                                ps[:, k0:k0 + kn], lw,
                                tiles[gl][:, c5 * DP + k0:c5 * DP + k0 + kn],
                                start=first, stop=last)

            # ---- chain ----
            ln_s = chain.tile([128, DP], F32, tag="ln_s")
            act(ln_s[:], sum_ps[:], AF.Ln)
            ln_pv = chain.tile([128, DP], F32, tag="ln_pv")
            act(ln_pv[:], epv_ps[:], AF.Ln)
            ce = chain.tile([128, DP], F16, tag="ce")
            nc.gpsimd.tensor_tensor(out=ce[:], in0=ln_s[:], in1=ln_pv[:],
                                    op=OP.subtract)
            eq = chain.tile([128, DP], F16, tag="eq")
            tt(out=eq[:], in0=epv_ps[:], in1=maxg_sg[:], op=OP.is_ge)
            iscr = chain.tile([128, DP], F16, tag="scr")
            nc.vector.scalar_tensor_tensor(
                out=iscr[:], in0=eq[:], scalar=0.0, in1=sw_t[:, d0:d0 + D],
                op0=OP.bypass, op1=OP.mult,
                accum_out=colstage[:, _col(sg, ph, 1):_col(sg, ph, 1) + 1])

            pt = chain.tile([128, DP], F16, tag="pt")
            act(pt[:], ce[:], AF.Exp, scale=-1.0)
            u = chain.tile([128, DP], F16, tag="u")
            act(u[:], pt[:], AF.Relu, bias=1.0, scale=-1.0)
            lu = chain.tile([128, DP], F16, tag="lu")
            act(lu[:], u[:], AF.Ln)
            p25 = chain.tile([128, DP], F16, tag="p25")
            act(p25[:], lu[:], AF.Exp, scale=2.5)
            fscr = chain.tile([128, DP], F16, tag="scr")
            nc.vector.scalar_tensor_tensor(
                out=fscr[:], in0=p25[:], scalar=0.0, in1=ce[:],
                op0=OP.bypass, op1=OP.mult,
                accum_out=colstage[:, _col(sg, ph, 0):_col(sg, ph, 0) + 1])

    dma(out=out_cols[:, :], in_=colstage[:])


_NC_CACHE = {}


def _get_nc():
    if "nc" not in _NC_CACHE:
        _NC_CACHE["nc"] = build_nc()
    return _NC_CACHE["nc"]


def _combine(res_list, pred, targets, inputs_arr, sf, ps, rd):
    B = pred.shape[0]
    t2 = targets.reshape(B, HW)

    fs = np.zeros(B, np.float64)
    iou_s = np.zeros(B, np.float64)
    for core, r in enumerate(res_list):
        cols = r["out_cols"].astype(np.float64)          # [128, NCOLS]
        for sg in range(NSG):
            rows = np.arange(128)
            bidx = core * B_PC + sg * 128 + rows
            f = sum(cols[:, _col(sg, ph, 0)] for ph in range(len(PHASES)))
            i = sum(cols[:, _col(sg, ph, 1)] for ph in range(len(PHASES)))
            fs[bidx] = f
            iou_s[bidx] = i

    # host-side stats from raw int inputs
    t3 = targets.reshape(B, H, W)
    trans = (t3[:, :, 1:] != t3[:, :, :-1]).sum((1, 2)) + \
            (t3[:, 1:, :] != t3[:, :-1, :]).sum((1, 2))
    off = (np.arange(B, dtype=np.int64)[:, None] * C)
    uniq = np.bincount((t2 + off).ravel(), minlength=B * C) \
        .reshape(B, C).astype(bool).sum(1)
    dcnt = (t2 == inputs_arr.reshape(B, HW)).sum(1)

    w = np.where(uniq > 4, 1.3, 1.0) * np.where(trans > W, 1.2, 1.0)
    focal = (fs * w).sum() / (B * HW)

    sw64 = _spatial_weights()
    SW = sw64.sum()
    # iou accumulates f16(sw) products; one flipped eq bit moves iou_s by
    # >= 1.0, so 0.5-tolerance compares on iou_s are exact bit tests
    sw16 = _spatial_weights().astype(np.float16).astype(np.float64).reshape(HW)
    strict = iou_s > sw16.sum() - 0.5
    iou = iou_s / SW
    ut = 0.85 * iou + 0.15 * strict
    ut_mean = ut.mean()
    exact_bonus = max(-ut_mean * 5.0, -5.0)

    iou_ti = ((t2 == inputs_arr.reshape(B, HW)) * sw16[None, :]).sum(1)
    cand = np.where(np.abs(iou_s - iou_ti) < 0.5)[0]
    copy = np.zeros(B, np.float64)
    if cand.size:
        pr = pred.reshape(B, C, HW)
        am = pr[cand].argmax(1)
        copy[cand] = (am == inputs_arr.reshape(B, HW)[cand]).all(1)
    transform_penalty = copy.mean() * 0.5

    sf_mean = sf.astype(np.float64).mean()
    creativity = 1.0 / (1.0 + np.exp(-sf_mean)) * 0.1
    strategic = ps.astype(np.float64).mean() * 0.1
    multi = rd.astype(np.float64).mean() * 0.1
    complexity = ut_mean * (HW / 1225.0) * 0.1

    total = (focal + transform_penalty + exact_bonus
             - creativity - strategic - multi - complexity)
    if np.isnan(total) or np.isinf(total):
        total = min(focal, 10.0)
    return np.float32(total)


def kernel(pred, strategic_features, planning_score, reasoning_depth,
           targets, inputs):
    global LAST_EXEC_NS
    pred = np.ascontiguousarray(np.asarray(pred, dtype=np.float32))
    targets = np.ascontiguousarray(np.asarray(targets, dtype=np.int32))
    inputs_arr = np.ascontiguousarray(np.asarray(inputs, dtype=np.int32))
    sf = np.asarray(strategic_features, dtype=np.float32)
    ps = np.asarray(planning_score, dtype=np.float32)
    rd = np.asarray(reasoning_depth, dtype=np.float32)

    nc = _get_nc()
    t16_full = targets.astype(np.float16)
    in_maps = []
    for core in range(N_CORES):
        sl = slice(core * B_PC, (core + 1) * B_PC)
        in_maps.append({
            "pred_in": pred[sl].reshape(B_PC, C, HW),
            "t_in": t16_full[sl].reshape(B_PC, HW),
        })

    trace = os.environ.get("BASSLOSS_TRACE", "0") == "1"
    res = run_bass_kernel_spmd(nc, in_maps, list(range(N_CORES)), trace=trace)
    LAST_EXEC_NS = res.exec_time_ns

    return _combine(res.results, pred, targets, inputs_arr, sf, ps, rd)


if __name__ == "__main__":
    d = np.load("/root/problem/inputs_cache.npz")
    out = kernel(**{k: d[k] for k in d.files})
    print("kernel out:", out, " exec_ns:", LAST_EXEC_NS)
